# revision 7
# baseline (speedup 1.0000x reference)
"""Trainium2 Bass kernel for the gnn_message_passing problem.

Strategy (8 NeuronCores, SPMD):
  - Vocab-shard the big embedding tables (img/txt/emb) row-wise across the
    8 cores.  Each core projects its 6250-row shard (img_table @ img_W + b,
    txt_table @ txt_W + b, item = emb + 0.1*img + 0.15*txt) on the PE.
    img/txt tables and weights are cast to bf16 on the host (halves HBM
    traffic; fp32 PSUM accumulation keeps error ~1e-3).
  - AllGather the projected tables (item fp32 [N,128], vis=img|txt bf16
    [N,256]) so each core holds the full padded tables in DRAM, with row 0
    reserved as the zero pad row (matches reference's `pad()` semantics).
  - Batch-shard the rest: each core handles 64 sessions (32 pairs).  Gathers
    (indirect DMA) pull the rows for session means and h0 = itemE[inputs].
    Session masked sums are computed with block-diagonal mask matmuls
    (2 sessions packed per matmul, K=100).  All session-level [B,128] math is
    done transposed ([128, 64], feature dim on partitions) so the 128x128
    weight matmuls need no transposes and biases are per-partition.
  - The 2 hypergraph layers run per session pair with block-diagonal
    Hs / Hs^T ([100,100], built on host - pure layout), degree scaling via
    per-partition activation scales, and the session-context injection via a
    K=2 indicator matmul.
"""

import sys

sys.path.insert(0, "/opt/trn_rl_repo")

import numpy as np
import ml_dtypes

import concourse.bass as bass
import concourse.bacc as bacc
import concourse.mybir as mybir
import concourse.tile as tile
from concourse import bass_utils

BF16 = ml_dtypes.bfloat16


class Cfg:
    def __init__(self, num_node=50000, dim=128, img_dim=1000, txt_dim=768,
                 batch=512, seq=50, ncore=8, vpad=None, cc=None):
        self.N = num_node
        self.D = dim
        self.IMG = img_dim
        self.TXT = txt_dim
        self.B = batch
        self.L = seq
        self.NC = ncore
        self.VS = self.N // self.NC              # raw rows per core
        # pad shard rows up to a multiple of 128
        self.VP = vpad if vpad else ((self.VS + 127) // 128) * 128
        assert self.VP % 128 == 0
        self.NF = 1 + self.NC * self.VP          # full padded table rows
        self.BS = self.B // self.NC              # sessions per core
        self.NPAIR = self.BS // 2
        self.L2 = 2 * self.L                     # pair-packed rows (100)
        assert self.L2 <= 128
        # img k-tiles: contraction split into <=128 chunks
        self.KI = (self.IMG + 127) // 128        # 8 tiles of 125
        self.KIW = self.IMG // self.KI           # 125
        assert self.KIW * self.KI == self.IMG
        self.KT = (self.TXT + 127) // 128        # 6 tiles of 128
        self.KTW = self.TXT // self.KT
        assert self.KTW * self.KT == self.TXT
        # outer chunking of the row dimension for phase A
        if cc is None:
            cc = 7 if (self.VP // 128) % 7 == 0 else 1
        self.CC = cc
        assert (self.VP // 128) % self.CC == 0
        self.CW = self.VP // self.CC             # chunk width in rows
        self.RPC = self.CW // 128                # 128-row tiles per chunk


REAL = Cfg()


def build_program(c: Cfg):
    f32 = mybir.dt.float32
    b16 = mybir.dt.bfloat16
    i32 = mybir.dt.int32
    AF = mybir.ActivationFunctionType
    AX = mybir.AxisListType
    OP = mybir.AluOpType

    nc = bacc.Bacc("TRN2", target_bir_lowering=False, debug=False,
                   num_devices=c.NC)

    def ein(nm, sh, dt):
        return nc.dram_tensor(nm, sh, dt, kind="ExternalInput")

    imgT = ein("imgT", [c.IMG, c.VP], b16)       # img table shard, transposed
    txtT = ein("txtT", [c.TXT, c.VP], b16)
    embS = ein("embS", [c.VP, c.D], f32)
    imgW = ein("imgW", [c.IMG, c.D], b16)
    txtW = ein("txtW", [c.TXT, c.D], b16)
    imgB = ein("imgB", [1, c.D], b16)
    txtB = ein("txtB", [1, c.D], b16)
    gvW = ein("gvW", [c.D, c.D], f32)
    gvB = ein("gvB", [c.D, 1], f32)
    gtW = ein("gtW", [c.D, c.D], f32)
    gtB = ein("gtB", [c.D, 1], f32)
    q1W = ein("q1W", [c.D, c.D], f32)
    q1B = ein("q1B", [c.D, 1], f32)
    q2W = ein("q2W", [c.D, 1], f32)
    Gbd = ein("Gbd", [c.NPAIR, c.L2, c.L2], f32)
    GTbd = ein("GTbd", [c.NPAIR, c.L2, c.L2], f32)
    Mbd32 = ein("Mbd32", [c.NPAIR, c.L2, 2], f32)
    Mbd16 = ein("Mbd16", [c.NPAIR, c.L2, 2], b16)
    maskT = ein("maskT", [c.L, c.BS], f32)
    ind2 = ein("ind2", [2, c.L2], f32)
    h0idx = ein("h0idx", [c.NPAIR, c.L2, 1], i32)
    ssidx = ein("ssidx", [c.NPAIR, c.L2, 1], i32)

    outH = nc.dram_tensor("outH", [c.BS, c.L, c.D], f32, kind="ExternalOutput")

    localI = nc.dram_tensor("localI", [c.VP, c.D], f32)
    localV = nc.dram_tensor("localV", [c.VP, 2 * c.D], b16)
    Titem = nc.dram_tensor("Titem", [c.NF, c.D], f32, addr_space="Shared")
    Tvis = nc.dram_tensor("Tvis", [c.NF, 2 * c.D], b16, addr_space="Shared")

    rg = [list(range(c.NC))]

    with tile.TileContext(nc) as tc:
        with (
            tc.tile_pool(name="wpool", bufs=1) as wp,
            tc.tile_pool(name="apool", bufs=2) as ap,
            tc.tile_pool(name="ostg", bufs=4) as ost,
            tc.tile_pool(name="cbig", bufs=1) as cb,
            tc.tile_pool(name="cgat", bufs=4) as cg,
            tc.tile_pool(name="csml", bufs=3) as cs,
        ):
            # ---- constant / weight tiles ----
            wi = [wp.tile([c.KIW, c.D], b16, tag=f"wi{k}", name=f"wi{k}") for k in range(c.KI)]
            for k in range(c.KI):
                nc.sync.dma_start(wi[k][:], imgW[k * c.KIW:(k + 1) * c.KIW, :])
            wt = [wp.tile([c.KTW, c.D], b16, tag=f"wt{k}", name=f"wt{k}") for k in range(c.KT)]
            for k in range(c.KT):
                nc.sync.dma_start(wt[k][:], txtW[k * c.KTW:(k + 1) * c.KTW, :])
            bi = wp.tile([1, c.D], b16, tag="bi")
            bt = wp.tile([1, c.D], b16, tag="bt")
            nc.sync.dma_start(bi[:], imgB[:])
            nc.sync.dma_start(bt[:], txtB[:])
            ones1 = wp.tile([1, c.D], b16, tag="ones1")
            nc.vector.memset(ones1[:], 1.0)

            # ---- zero row 0 of the full tables ----
            zi = wp.tile([1, c.D], f32, tag="zi")
            zv = wp.tile([1, 2 * c.D], b16, tag="zv")
            nc.vector.memset(zi[:], 0.0)
            nc.vector.memset(zv[:], 0.0)
            nc.sync.dma_start(Titem[0:1, :], zi[:])
            nc.sync.dma_start(Tvis[0:1, :], zv[:])

            # ================= Phase A: projections =================
            imgT3 = imgT.rearrange("(k f) v -> f k v", k=c.KI)
            txtT3 = txtT.rearrange("(k f) v -> f k v", k=c.KT)
            psA_ctx = tc.tile_pool(name="psA", bufs=2, space="PSUM")
            psA = psA_ctx.__enter__()
            for ccx in range(c.CC):
                cw0 = ccx * c.CW
                ai = ap.tile([c.KIW, c.KI * c.CW], b16, tag="ai")
                nc.sync.dma_start(
                    ai[:].rearrange("f (k v) -> f k v", k=c.KI),
                    imgT3[:, :, cw0:cw0 + c.CW])
                at = ap.tile([c.KTW, c.KT * c.CW], b16, tag="at")
                nc.sync.dma_start(
                    at[:].rearrange("f (k v) -> f k v", k=c.KT),
                    txtT3[:, :, cw0:cw0 + c.CW])
                ae = ap.tile([128, c.RPC * c.D], f32, tag="ae")
                nc.sync.dma_start(
                    ae[:].rearrange("p (n d) -> p n d", n=c.RPC),
                    embS.rearrange("(n p) d -> p n d", p=128)[
                        :, ccx * c.RPC:(ccx + 1) * c.RPC, :])
                for r2 in range(c.RPC):
                    ps_i = psA.tile([128, c.D], f32, tag="psi")
                    for k in range(c.KI):
                        nc.tensor.matmul(
                            ps_i[:], lhsT=ai[:, k * c.CW + r2 * 128:
                                             k * c.CW + r2 * 128 + 128],
                            rhs=wi[k][:], start=(k == 0), stop=False)
                    nc.tensor.matmul(ps_i[:], lhsT=ones1[:], rhs=bi[:],
                                     start=False, stop=True)
                    ps_t = psA.tile([128, c.D], f32, tag="pst")
                    for k in range(c.KT):
                        nc.tensor.matmul(
                            ps_t[:], lhsT=at[:, k * c.CW + r2 * 128:
                                             k * c.CW + r2 * 128 + 128],
                            rhs=wt[k][:], start=(k == 0), stop=False)
                    nc.tensor.matmul(ps_t[:], lhsT=ones1[:], rhs=bt[:],
                                     start=False, stop=True)
                    # vis out (bf16): [128, 256] = img | txt
                    vo = ost.tile([128, 2 * c.D], b16, tag="vo")
                    nc.scalar.copy(vo[:, 0:c.D], ps_i[:])
                    nc.scalar.copy(vo[:, c.D:2 * c.D], ps_t[:])
                    # item out (fp32): emb + 0.1 img + 0.15 txt
                    t1 = ost.tile([128, c.D], f32, tag="t1")
                    nc.vector.tensor_scalar_mul(t1[:], ps_i[:], 0.1)
                    t2 = ost.tile([128, c.D], f32, tag="t2")
                    nc.vector.tensor_scalar_mul(t2[:], ps_t[:], 0.15)
                    io = ost.tile([128, c.D], f32, tag="io")
                    nc.vector.tensor_add(io[:], t1[:], t2[:])
                    nc.vector.tensor_add(
                        io[:], io[:], ae[:, r2 * c.D:(r2 + 1) * c.D])
                    row0 = cw0 + r2 * 128
                    nc.sync.dma_start(localV[row0:row0 + 128, :], vo[:])
                    nc.sync.dma_start(localI[row0:row0 + 128, :], io[:])

            psA_ctx.__exit__(None, None, None)

            # ================= Phase B: all-gather =================
            nc.gpsimd.collective_compute(
                "AllGather", mybir.AluOpType.bypass, replica_groups=rg,
                ins=[localI[:].opt()], outs=[Titem[1:c.NF, :].opt()])
            nc.gpsimd.collective_compute(
                "AllGather", mybir.AluOpType.bypass, replica_groups=rg,
                ins=[localV[:].opt()], outs=[Tvis[1:c.NF, :].opt()])

            # ================= Phase C: per-batch-shard =================
            # persistent loads
            h0all = cb.tile([c.L2, c.NPAIR * c.D], f32, tag="h0all")
            Gsb = cb.tile([c.L2, c.NPAIR * c.L2], f32, tag="Gsb")
            GTsb = cb.tile([c.L2, c.NPAIR * c.L2], f32, tag="GTsb")
            nc.sync.dma_start(
                Gsb[:].rearrange("l (p e) -> l p e", p=c.NPAIR),
                Gbd.rearrange("p l e -> l p e"))
            nc.sync.dma_start(
                GTsb[:].rearrange("l (p e) -> l p e", p=c.NPAIR),
                GTbd.rearrange("p l e -> l p e"))
            m32 = cb.tile([c.L2, c.NPAIR * 2], f32, tag="m32")
            m16 = cb.tile([c.L2, c.NPAIR * 2], b16, tag="m16")
            nc.sync.dma_start(
                m32[:].rearrange("l (p j) -> l p j", p=c.NPAIR),
                Mbd32.rearrange("p l j -> l p j"))
            nc.sync.dma_start(
                m16[:].rearrange("l (p j) -> l p j", p=c.NPAIR),
                Mbd16.rearrange("p l j -> l p j"))
            hix = cb.tile([c.L2, c.NPAIR], i32, tag="hix")
            six = cb.tile([c.L2, c.NPAIR], i32, tag="six")
            nc.sync.dma_start(hix[:], h0idx.rearrange("p l o -> l (p o)"))
            nc.sync.dma_start(six[:], ssidx.rearrange("p l o -> l (p o)"))
            mkT = cb.tile([c.L, c.BS], f32, tag="mkT")
            nc.sync.dma_start(mkT[:], maskT[:])
            i2 = cb.tile([2, c.L2], f32, tag="i2")
            nc.sync.dma_start(i2[:], ind2[:])
            ones50 = cb.tile([c.L, 1], f32, tag="ones50")
            nc.vector.memset(ones50[:], 1.0)
            # session weights
            wgv = cb.tile([c.D, c.D], f32, tag="wgv")
            wgt = cb.tile([c.D, c.D], f32, tag="wgt")
            wq1 = cb.tile([c.D, c.D], f32, tag="wq1")
            wq2 = cb.tile([c.D, 1], f32, tag="wq2")
            bgv = cb.tile([c.D, 1], f32, tag="bgv")
            bgt = cb.tile([c.D, 1], f32, tag="bgt")
            bq1 = cb.tile([c.D, 1], f32, tag="bq1")
            nc.sync.dma_start(wgv[:], gvW[:])
            nc.sync.dma_start(wgt[:], gtW[:])
            nc.sync.dma_start(wq1[:], q1W[:])
            nc.sync.dma_start(wq2[:], q2W[:])
            nc.sync.dma_start(bgv[:], gvB[:])
            nc.sync.dma_start(bgt[:], gtB[:])
            nc.sync.dma_start(bq1[:], q1B[:])

            # ---- C1: gathers + masked sums ----
            Xim = cb.tile([c.D, c.BS], f32, tag="Xim")
            Xtx = cb.tile([c.D, c.BS], f32, tag="Xtx")
            Xit = cb.tile([c.D, c.BS], f32, tag="Xit")
            with tc.tile_pool(name="psm", bufs=2, space="PSUM") as psm:
                for p in range(c.NPAIR):
                    gv_ = cg.tile([c.L2, 2 * c.D], b16, tag="gvis")
                    nc.gpsimd.indirect_dma_start(
                        out=gv_[:], out_offset=None, in_=Tvis[:],
                        in_offset=bass.IndirectOffsetOnAxis(
                            ap=six[:, p:p + 1], axis=0))
                    gi_ = cg.tile([c.L2, c.D], f32, tag="gitm")
                    nc.gpsimd.indirect_dma_start(
                        out=gi_[:], out_offset=None, in_=Titem[:],
                        in_offset=bass.IndirectOffsetOnAxis(
                            ap=six[:, p:p + 1], axis=0))
                    # h0 gather (kept in SBUF for phase C3)
                    nc.gpsimd.indirect_dma_start(
                        out=h0all[:, p * c.D:(p + 1) * c.D], out_offset=None,
                        in_=Titem[:],
                        in_offset=bass.IndirectOffsetOnAxis(
                            ap=hix[:, p:p + 1], axis=0))
                    pim = psm.tile([c.D, 2], f32, tag="pim")
                    nc.tensor.matmul(pim[:], lhsT=gv_[:, 0:c.D],
                                     rhs=m16[:, 2 * p:2 * p + 2],
                                     start=True, stop=True)
                    ptx = psm.tile([c.D, 2], f32, tag="ptx")
                    nc.tensor.matmul(ptx[:], lhsT=gv_[:, c.D:2 * c.D],
                                     rhs=m16[:, 2 * p:2 * p + 2],
                                     start=True, stop=True)
                    pit = psm.tile([c.D, 2], f32, tag="pit")
                    nc.tensor.matmul(pit[:], lhsT=gi_[:],
                                     rhs=m32[:, 2 * p:2 * p + 2],
                                     start=True, stop=True)
                    nc.scalar.copy(Xim[:, 2 * p:2 * p + 2], pim[:])
                    nc.scalar.copy(Xtx[:, 2 * p:2 * p + 2], ptx[:])
                    nc.scalar.copy(Xit[:, 2 * p:2 * p + 2], pit[:])

            # ---- C2: session fusion math (transposed [128, BS]) ----
            with (
                tc.tile_pool(name="psq", bufs=1, space="PSUM") as psq,
                tc.tile_pool(name="psg", bufs=2, space="PSUM") as psg,
            ):
                dT = psq.tile([1, c.BS], f32, tag="dT")
                nc.tensor.matmul(dT[:], lhsT=ones50[:], rhs=mkT[:],
                                 start=True, stop=True)
                invd = cs.tile([1, c.BS], f32, tag="invd")
                nc.vector.reciprocal(invd[:], dT[:])
                onesf = cb.tile([1, c.D], f32, tag="onesf")
                nc.vector.memset(onesf[:], 1.0)

                def rep_row(row):
                    # replicate a [1, BS] row across all D partitions (PSUM)
                    rp = psg.tile([c.D, c.BS], f32, tag="rep", name="rp")
                    nc.tensor.matmul(rp[:], lhsT=onesf[:], rhs=row,
                                     start=True, stop=True)
                    return rp

                Xim_m = cb.tile([c.D, c.BS], f32, tag="Xim_m")
                Xtx_m = cb.tile([c.D, c.BS], f32, tag="Xtx_m")
                Xit_m = cb.tile([c.D, c.BS], f32, tag="Xit_m")
                ir = rep_row(invd[:])
                nc.vector.tensor_tensor(Xim_m[:], Xim[:], ir[:], op=OP.mult)
                nc.vector.tensor_tensor(Xtx_m[:], Xtx[:], ir[:], op=OP.mult)
                nc.vector.tensor_tensor(Xit_m[:], Xit[:], ir[:], op=OP.mult)

                # gates on 2*session_img / 2*session_txt (scale=2 in ACT)
                pgv = psg.tile([c.D, c.BS], f32, tag="pg")
                nc.tensor.matmul(pgv[:], lhsT=wgv[:], rhs=Xim_m[:],
                                 start=True, stop=True)
                gv1 = cs.tile([c.D, c.BS], f32, tag="gv1")
                nc.scalar.activation(gv1[:], pgv[:], AF.Sigmoid,
                                     bias=bgv[:, :1], scale=2.0)
                pgt = psg.tile([c.D, c.BS], f32, tag="pg")
                nc.tensor.matmul(pgt[:], lhsT=wgt[:], rhs=Xtx_m[:],
                                 start=True, stop=True)
                gt1 = cs.tile([c.D, c.BS], f32, tag="gt1")
                nc.scalar.activation(gt1[:], pgt[:], AF.Sigmoid,
                                     bias=bgt[:, :1], scale=2.0)
                sid = cb.tile([c.D, c.BS], f32, tag="sid")
                std = cb.tile([c.D, c.BS], f32, tag="std")
                nc.vector.tensor_mul(sid[:], Xit_m[:], gv1[:])
                nc.vector.tensor_mul(std[:], Xit_m[:], gt1[:])

                # qc scores
                def qc(xin, tag):
                    pq = psg.tile([c.D, c.BS], f32, tag="pg")
                    nc.tensor.matmul(pq[:], lhsT=wq1[:], rhs=xin[:],
                                     start=True, stop=True)
                    th = cs.tile([c.D, c.BS], f32, tag="th")
                    nc.scalar.activation(th[:], pq[:], AF.Tanh,
                                         bias=bq1[:, :1], scale=1.0)
                    qq = psq.tile([1, c.BS], f32, tag="qq" + tag)
                    nc.tensor.matmul(qq[:], lhsT=wq2[:], rhs=th[:],
                                     start=True, stop=True)
                    return qq

                q1p = qc(sid, "a")
                q2p = qc(std, "b")
                q1v = cs.tile([1, c.BS], f32, tag="q1v")
                q2v = cs.tile([1, c.BS], f32, tag="q2v")
                nc.vector.tensor_copy(q1v[:], q1p[:])
                nc.vector.tensor_copy(q2v[:], q2p[:])
                qm = cs.tile([1, c.BS], f32, tag="qm")
                nc.vector.tensor_tensor(qm[:], q1v[:], q2v[:], op=OP.max)
                e1 = cs.tile([1, c.BS], f32, tag="e1")
                e2 = cs.tile([1, c.BS], f32, tag="e2")
                nc.vector.tensor_sub(e1[:], q1v[:], qm[:])
                nc.vector.tensor_sub(e2[:], q2v[:], qm[:])
                nc.scalar.activation(e1[:], e1[:], AF.Exp)
                nc.scalar.activation(e2[:], e2[:], AF.Exp)
                esum = cs.tile([1, c.BS], f32, tag="esum")
                nc.vector.tensor_add(esum[:], e1[:], e2[:])
                rsum = cs.tile([1, c.BS], f32, tag="rsum")
                nc.vector.reciprocal(rsum[:], esum[:])
                w1 = cs.tile([1, c.BS], f32, tag="w1")
                w2 = cs.tile([1, c.BS], f32, tag="w2")
                nc.vector.tensor_mul(w1[:], e1[:], rsum[:])
                nc.vector.tensor_mul(w2[:], e2[:], rsum[:])

                com = cb.tile([c.D, c.BS], f32, tag="com")
                tmp1 = cs.tile([c.D, c.BS], f32, tag="tmp1")
                w1r = rep_row(w1[:])
                nc.vector.tensor_tensor(com[:], sid[:], w1r[:], op=OP.mult)
                w2r = rep_row(w2[:])
                nc.vector.tensor_tensor(tmp1[:], std[:], w2r[:], op=OP.mult)
                nc.vector.tensor_add(com[:], com[:], tmp1[:])

                # gates on session_item
                pg2 = psg.tile([c.D, c.BS], f32, tag="pg")
                nc.tensor.matmul(pg2[:], lhsT=wgv[:], rhs=Xit_m[:],
                                 start=True, stop=True)
                gv2 = cs.tile([c.D, c.BS], f32, tag="gv2")
                nc.scalar.activation(gv2[:], pg2[:], AF.Sigmoid,
                                     bias=bgv[:, :1], scale=1.0)
                pg3 = psg.tile([c.D, c.BS], f32, tag="pg")
                nc.tensor.matmul(pg3[:], lhsT=wgt[:], rhs=Xit_m[:],
                                 start=True, stop=True)
                gt2 = cs.tile([c.D, c.BS], f32, tag="gt2")
                nc.scalar.activation(gt2[:], pg3[:], AF.Sigmoid,
                                     bias=bgt[:, :1], scale=1.0)

                sep = cs.tile([c.D, c.BS], f32, tag="sep")
                nc.vector.tensor_sub(sep[:], sid[:], com[:])
                nc.vector.tensor_mul(sep[:], gv2[:], sep[:])
                sep2 = cs.tile([c.D, c.BS], f32, tag="sep2")
                nc.vector.tensor_sub(sep2[:], std[:], com[:])
                nc.vector.tensor_mul(sep2[:], gt2[:], sep2[:])
                fus = cs.tile([c.D, c.BS], f32, tag="fus")
                nc.vector.tensor_add(fus[:], sep[:], sep2[:])
                nc.vector.tensor_add(fus[:], fus[:], com[:])
                nc.vector.tensor_scalar_mul(fus[:], fus[:], 1.0 / 3.0)
                # session_diff = item + img + txt + fusion  (transposed)
                Xs = cb.tile([c.D, c.BS], f32, tag="Xs")
                nc.vector.tensor_add(Xs[:], Xit_m[:], Xim_m[:])
                nc.vector.tensor_add(Xs[:], Xs[:], Xtx_m[:])
                nc.vector.tensor_add(Xs[:], Xs[:], fus[:])

            # identity for PE transposes
            ident = cb.tile([128, 128], f32, tag="ident")
            from concourse.masks import make_identity
            make_identity(nc, ident[:])

            # ---- C3: hypergraph layers per pair ----
            with (
                tc.tile_pool(name="psT", bufs=2, space="PSUM") as psT,
                tc.tile_pool(name="psR", bufs=2, space="PSUM") as psR,
                tc.tile_pool(name="psE", bufs=2, space="PSUM") as psE,
            ):
                for p in range(c.NPAIR):
                    Gp = Gsb[:, p * c.L2:(p + 1) * c.L2]
                    GTp = GTsb[:, p * c.L2:(p + 1) * c.L2]
                    dgn = cs.tile([c.L2, 1], f32, tag="dgn")
                    dge = cs.tile([c.L2, 1], f32, tag="dge")
                    nc.vector.reduce_sum(dgn[:], Gp, axis=AX.X)
                    nc.vector.reduce_sum(dge[:], GTp, axis=AX.X)
                    idn = cs.tile([c.L2, 1], f32, tag="idn")
                    ide = cs.tile([c.L2, 1], f32, tag="ide")
                    nc.vector.reciprocal(idn[:], dgn[:])
                    nc.vector.reciprocal(ide[:], dge[:])
                    # s pair rows: transpose Xs[:, 2p:2p+2] -> [2, 128]
                    tp = psT.tile([2, c.D], f32, tag="tp")
                    nc.tensor.transpose(tp[:], Xs[:, 2 * p:2 * p + 2],
                                        ident[:])
                    sp = cs.tile([2, c.D], f32, tag="sp")
                    nc.vector.tensor_copy(sp[:], tp[:])
                    srep = psR.tile([c.L2, c.D], f32, tag="srep")
                    nc.tensor.matmul(srep[:], lhsT=i2[:], rhs=sp[:],
                                     start=True, stop=True)
                    hcur = h0all[:, p * c.D:(p + 1) * c.D]
                    for lyr in range(2):
                        pe_ = psE.tile([c.L2, c.D], f32, tag="pe")
                        nc.tensor.matmul(pe_[:], lhsT=Gp, rhs=hcur,
                                         start=True, stop=True)
                        ee = cs.tile([c.L2, c.D], f32, tag="ee")
                        nc.scalar.activation(ee[:], pe_[:], AF.Copy,
                                             scale=ide[:, :1])
                        ph_ = psE.tile([c.L2, c.D], f32, tag="ph")
                        nc.tensor.matmul(ph_[:], lhsT=GTp, rhs=ee[:],
                                         start=True, stop=True)
                        hh = cs.tile([c.L2, c.D], f32, tag=f"hh{lyr}")
                        nc.scalar.activation(hh[:], ph_[:], AF.Copy,
                                             scale=idn[:, :1])
                        nc.vector.tensor_add(hh[:], hh[:], srep[:])
                        hcur = hh[:]
                    nc.sync.dma_start(
                        outH[2 * p:2 * p + 2].rearrange("b l d -> (b l) d"),
                        hcur)

    nc.compile()
    return nc


_CACHE = {}


def _get_program(c: Cfg):
    key = (c.N, c.B)
    if key not in _CACHE:
        _CACHE[key] = build_program(c)
    return _CACHE[key]


def _dev_row(v, c: Cfg):
    """Map a reference index (0 = pad row) to a padded-table device row."""
    v = np.asarray(v, dtype=np.int64)
    r = v - 1
    out = 1 + (r // c.VS) * c.VP + (r % c.VS)
    return np.where(v == 0, 0, out).astype(np.int32)


def _prep_inputs(c: Cfg, inputs, item, mask_item, Hs, emb_table, img_table,
                 txt_table, img_W, img_b, txt_W, txt_b, gate_v_W, gate_v_b,
                 gate_t_W, gate_t_b, qc_W1, qc_b1, qc_W2):
    f32 = np.float32
    imgT = np.zeros((c.IMG, c.NC * c.VP), dtype=BF16)
    txtT = np.zeros((c.TXT, c.NC * c.VP), dtype=BF16)
    embS = np.zeros((c.NC * c.VP, c.D), dtype=f32)
    imgTt = np.ascontiguousarray(img_table.T).astype(BF16)
    txtTt = np.ascontiguousarray(txt_table.T).astype(BF16)
    for k in range(c.NC):
        imgT[:, k * c.VP:k * c.VP + c.VS] = imgTt[:, k * c.VS:(k + 1) * c.VS]
        txtT[:, k * c.VP:k * c.VP + c.VS] = txtTt[:, k * c.VS:(k + 1) * c.VS]
        embS[k * c.VP:k * c.VP + c.VS] = emb_table[k * c.VS:(k + 1) * c.VS]

    maskf = mask_item.astype(f32)
    in_maps = []
    for k in range(c.NC):
        b0, b1 = k * c.BS, (k + 1) * c.BS
        Hk = Hs[b0:b1].astype(f32)
        mk = maskf[b0:b1]
        Gbd = np.zeros((c.NPAIR, c.L2, c.L2), f32)
        GTbd = np.zeros((c.NPAIR, c.L2, c.L2), f32)
        Mbd = np.zeros((c.NPAIR, c.L2, 2), f32)
        for p in range(c.NPAIR):
            Gbd[p, :c.L, :c.L] = Hk[2 * p]
            Gbd[p, c.L:, c.L:] = Hk[2 * p + 1]
            GTbd[p, :c.L, :c.L] = Hk[2 * p].T
            GTbd[p, c.L:, c.L:] = Hk[2 * p + 1].T
            Mbd[p, :c.L, 0] = mk[2 * p]
            Mbd[p, c.L:, 1] = mk[2 * p + 1]
        ind2 = np.zeros((2, c.L2), f32)
        ind2[0, :c.L] = 1.0
        ind2[1, c.L:] = 1.0
        in_maps.append({
            "imgT": imgT[:, k * c.VP:(k + 1) * c.VP],
            "txtT": txtT[:, k * c.VP:(k + 1) * c.VP],
            "embS": embS[k * c.VP:(k + 1) * c.VP],
            "imgW": img_W.astype(BF16),
            "txtW": txt_W.astype(BF16),
            "imgB": img_b.reshape(1, c.D).astype(BF16),
            "txtB": txt_b.reshape(1, c.D).astype(BF16),
            "gvW": gate_v_W.astype(f32), "gvB": gate_v_b.reshape(c.D, 1).astype(f32),
            "gtW": gate_t_W.astype(f32), "gtB": gate_t_b.reshape(c.D, 1).astype(f32),
            "q1W": qc_W1.astype(f32), "q1B": qc_b1.reshape(c.D, 1).astype(f32),
            "q2W": qc_W2.astype(f32),
            "Gbd": Gbd, "GTbd": GTbd,
            "Mbd32": Mbd, "Mbd16": Mbd.astype(BF16),
            "maskT": np.ascontiguousarray(mk.T),
            "ind2": ind2,
            "h0idx": _dev_row(inputs[b0:b1], c).reshape(c.NPAIR, c.L2, 1),
            "ssidx": _dev_row(item[b0:b1], c).reshape(c.NPAIR, c.L2, 1),
        })
    return in_maps


def run(c: Cfg, trace=False, **inputs):
    nc = _get_program(c)
    in_maps = _prep_inputs(c, **{k: np.asarray(v) for k, v in inputs.items()})
    res = bass_utils.run_bass_kernel_spmd(
        nc, in_maps, core_ids=list(range(c.NC)), trace=trace)
    out = np.concatenate([r["outH"] for r in res.results], axis=0)
    return out.astype(np.float32), res


def kernel(**inputs):
    out, _ = run(REAL, trace=False, **inputs)
    return out
